# revision 1
# baseline (speedup 1.0000x reference)
"""Trainium2 Bass kernel for nn_Conv_39273180955616.

Computes, for X:(16,64,512,512) f32, K:(1,1,7,7), b:(1,1,1,1):
    out[n,c] = correlate2d(X[n,c], Keff, pad=3) + 49*b
where Keff = K.sum(axis=(0,1)).

Strategy: pure data parallel over the 1024 (n,c) planes -> 128 planes/core
on 8 cores.  Per plane, the 7x7 correlation runs on TensorE as
banded-Toeplitz matmuls: the h-dimension contraction is a [K<=128, 128]
band matrix (7 diagonals of one kernel column) against an image block
(rows on partitions), and the 7 w-shifts are free-dim offsets into a
zero-padded (W+6) image row, accumulated in PSUM.  The 24-row bottom
tiles of 4 consecutive planes are packed into one block-diagonal matmul
set (stacked on partitions), cutting the matmul count by 15%.  Inputs
are pre-cast to bf16 on host (PSUM accumulates in fp32); bias is added
during PSUM->SBUF eviction, alternating ScalarE/VectorE.  DMA is
batched and spread across the SP-HWDGE and SWDGE rings.
"""
import numpy as np
import ml_dtypes

import concourse.bass as bass
import concourse.tile as tile
from concourse import bacc, mybir
from concourse.bass_utils import run_bass_kernel_spmd

N_CORES = 8
H = 512
W = 512
WPAD = W + 6  # 3 zero columns each side
N_PLANES_TOTAL = 16 * 64
PLANES_PER_CORE = N_PLANES_TOTAL // N_CORES  # 128
GROUP = 4  # planes per bottom-tile merge group

# Per-plane tiles: 4 x 122 output rows (kinds 0/1); the 24-row bottom
# tile (kind 2) is handled once per GROUP planes as a block-diagonal
# [108, 96] matmul (4 x K=27 / M=24 blocks stacked on partitions).
# (out_row0, out_rows, in_row0, in_rows, kind)
TILES = [
    (0, 122, 0, 125, 0),
    (122, 122, 119, 128, 1),
    (244, 122, 241, 128, 1),
    (366, 122, 363, 128, 1),
]
KIND_K = {0: 125, 1: 128, 2: GROUP * 27}
M_PAD = 128  # lhsT padded to 128 cols -> FWL eligible; pad rows are zero
WCOLS = 3 * 7 * M_PAD


def _build_weight_pack(Keff: np.ndarray) -> np.ndarray:
    """Keff (7,7) f32 -> packed banded-Toeplitz lhsT matrices [128, WCOLS] bf16.

    Matrix for (kind, dw) sits at cols [(kind*7+dw)*128, ...+128).
    lhsT[p, m] = Keff[dh, dw], dh = p - m (+3 for kind 0); matmul computes
    out[m, w] = sum_p lhsT[p, m] * block[p, w + dw].  Kind 2 is the
    block-diagonal stack of GROUP bottom tiles: block g at rows
    [27g, 27g+27) x cols [24g, 24g+24).
    """
    wp = np.zeros((128, WCOLS), np.float32)
    for kind in (0, 1):
        Kk = KIND_K[kind]
        p = np.arange(Kk)[:, None]
        m = np.arange(122)[None, :]
        dh = p - m + (3 if kind == 0 else 0)
        ok = (dh >= 0) & (dh < 7)
        for dw in range(7):
            mat = np.zeros((Kk, M_PAD), np.float32)
            mat[:, :122][ok] = Keff[dh[ok], dw]
            c0 = (kind * 7 + dw) * M_PAD
            wp[:Kk, c0:c0 + M_PAD] = mat
    # kind 2 block-diagonal
    p = np.arange(27)[:, None]
    m = np.arange(24)[None, :]
    dh = p - m
    ok = (dh >= 0) & (dh < 7)
    for dw in range(7):
        blk = np.zeros((27, 24), np.float32)
        blk[ok] = Keff[dh[ok], dw]
        c0 = (2 * 7 + dw) * M_PAD
        for g in range(GROUP):
            wp[27 * g:27 * g + 27, c0 + 24 * g:c0 + 24 * g + 24] = blk
    return wp.astype(ml_dtypes.bfloat16)


_NC_CACHE = {}


def _get_module(n_planes: int):
    if n_planes in _NC_CACHE:
        return _NC_CACHE[n_planes]
    assert n_planes % GROUP == 0
    nc = bacc.Bacc("TRN2", target_bir_lowering=False, debug=False,
                   num_devices=N_CORES)
    xp = nc.dram_tensor("xp", [n_planes, H, WPAD], mybir.dt.bfloat16,
                        kind="ExternalInput")
    wt = nc.dram_tensor("wt", [128, WCOLS], mybir.dt.bfloat16,
                        kind="ExternalInput")
    bv = nc.dram_tensor("bv", [128, 1], mybir.dt.float32,
                        kind="ExternalInput")
    out = nc.dram_tensor("out", [n_planes, H, W], mybir.dt.float32,
                         kind="ExternalOutput")

    x_elems = H * WPAD  # per-plane element count in xp

    with tile.TileContext(nc) as tc:
        with (
            tc.tile_pool(name="wp", bufs=1) as wpool,
            tc.tile_pool(name="xa", bufs=8) as xapool,
            tc.tile_pool(name="xb", bufs=8) as xbpool,
            tc.tile_pool(name="xg", bufs=3) as xgpool,
            tc.tile_pool(name="ps", bufs=8, space="PSUM") as pspool,
            tc.tile_pool(name="ob", bufs=10) as obpool,
            tc.tile_pool(name="og", bufs=3) as ogpool,
        ):
            wtile = wpool.tile([128, WCOLS], mybir.dt.bfloat16)
            nc.sync.dma_start(wtile[:], wt.ap())
            btile = wpool.tile([128, 1], mybir.dt.float32)
            nc.sync.dma_start(btile[:], bv.ap())

            def evict(engine, dst, src, rows):
                if engine == "act":
                    nc.scalar.activation(
                        dst, src, mybir.ActivationFunctionType.Identity,
                        bias=btile[:rows, :], scale=1.0)
                else:
                    nc.vector.tensor_scalar_add(dst, src, btile[:rows, :])

            for g0 in range(0, n_planes, GROUP):
                # bottom rows (485..511) of GROUP planes in one load
                xg = xgpool.tile([GROUP * 27, WPAD], mybir.dt.bfloat16)
                for g in range(GROUP):
                    nc.sync.dma_start(
                        xg[27 * g:27 * g + 27, :],
                        bass.AP(xp, (g0 + g) * x_elems + 485 * WPAD,
                                [[WPAD, 27], [1, WPAD]]))
                for p in range(g0, g0 + GROUP):
                    # ---- input loads (SP ring) ----
                    xa = xapool.tile([125, WPAD], mybir.dt.bfloat16)
                    nc.sync.dma_start(
                        xa[:], bass.AP(xp, p * x_elems,
                                       [[WPAD, 125], [1, WPAD]]))
                    xb = xbpool.tile([128, 3 * WPAD], mybir.dt.bfloat16)
                    # rows 119+122b+q, b=0..2 (overlapping strided read)
                    nc.sync.dma_start(
                        xb[:].rearrange("p (b w) -> p b w", b=3),
                        bass.AP(xp, p * x_elems + 119 * WPAD,
                                [[WPAD, 128], [122 * WPAD, 3], [1, WPAD]]))

                    ob = obpool.tile([122, 4 * W], mybir.dt.float32)
                    for t, (or0, oh, ir0, ih, kind) in enumerate(TILES):
                        if kind == 0:
                            rhs_of = lambda dw: xa[:, dw:dw + W]
                        else:
                            b = t - 1
                            rhs_of = lambda dw, b=b: xb[:, b * WPAD + dw:
                                                        b * WPAD + dw + W]
                        pt = pspool.tile([128, W], mybir.dt.float32)
                        for dw in range(7):
                            c0 = (kind * 7 + dw) * M_PAD
                            nc.tensor.matmul(
                                pt[:, :], wtile[:ih, c0:c0 + M_PAD],
                                rhs_of(dw), start=(dw == 0), stop=(dw == 6))
                        evict("act" if t % 2 == 0 else "dve",
                              ob[:, t * W:(t + 1) * W], pt[:122, :], 122)
                    # rows 0..487 = 4 tiles of 122 (1 MB); alternate the
                    # SWDGE and ACT-HWDGE rings so store completions keep up
                    store_eng = nc.gpsimd if p % 2 == 0 else nc.scalar
                    store_eng.dma_start(
                        bass.AP(out, p * H * W,
                                [[W, 122], [122 * W, 4], [1, W]]),
                        ob[:].rearrange("p (b w) -> p b w", b=4))

                # ---- merged bottom tiles of the group ----
                pt = pspool.tile([128, W], mybir.dt.float32)
                for dw in range(7):
                    c0 = (2 * 7 + dw) * M_PAD
                    nc.tensor.matmul(
                        pt[:, :], wtile[:GROUP * 27, c0:c0 + M_PAD],
                        xg[:, dw:dw + W], start=(dw == 0), stop=(dw == 6))
                og = ogpool.tile([GROUP * 24, W], mybir.dt.float32)
                evict("act", og[:], pt[:GROUP * 24, :], GROUP * 24)
                for g in range(GROUP):
                    nc.gpsimd.dma_start(
                        bass.AP(out, ((g0 + g) * H + 488) * W,
                                [[W, 24], [1, W]]),
                        og[24 * g:24 * g + 24, :])

    nc.compile()
    _NC_CACHE[n_planes] = nc
    return nc


def _prep_inputs(X, K, b, n_cores=N_CORES):
    Keff = np.asarray(K, np.float32).sum(axis=(0, 1))
    wt = _build_weight_pack(Keff)
    bias = np.float32(np.asarray(b).reshape(-1)[0]) * np.float32(K.size)
    bv = np.full((128, 1), bias, np.float32)

    Xr = np.asarray(X, np.float32).reshape(-1, H, W)
    n_total = Xr.shape[0]
    per = n_total // n_cores
    Xp = np.zeros((n_total, H, WPAD), ml_dtypes.bfloat16)
    Xp[:, :, 3:3 + W] = Xr.astype(ml_dtypes.bfloat16)
    in_maps = [
        {"xp": Xp[i * per:(i + 1) * per], "wt": wt, "bv": bv}
        for i in range(n_cores)
    ]
    return in_maps, per


def kernel(X, K, b):
    in_maps, per = _prep_inputs(X, K, b)
    nc = _get_module(per)
    res = run_bass_kernel_spmd(nc, in_maps, list(range(N_CORES)))
    out = np.concatenate([res.results[i]["out"] for i in range(N_CORES)], axis=0)
    return out.reshape(np.asarray(X).shape)



# revision 5
# speedup vs baseline: 1.6117x; 1.6117x over previous
"""Trainium2 Bass kernel for nn_Conv_39273180955616.

Computes, for X:(16,64,512,512) f32, K:(1,1,7,7), b:(1,1,1,1):
    out[n,c] = correlate2d(X[n,c], Keff, pad=3) + 49*b
where Keff = K.sum(axis=(0,1)).

Strategy: pure data parallel over the 1024 (n,c) planes -> 128 planes/core
on 8 cores.  The 7x7 correlation runs on TensorE as banded-Toeplitz
matmuls in fp8-e4m3 DoubleRow mode (2 MACs/cell/cycle): contraction
pairs (p,i) map to image-row pairs, so a [64x2, 122] band plus the
7 w-shifts (free-dim offsets into the zero-padded row) computes a
122-row output tile in 7 N=512 matmuls of 256 cycles each.

fp8 quantization error is compensated with error feedback: the unused
half of the contraction dim (partitions 64..127) carries E8 =
fp8(X - fp8(X)) convolved with the same band, cancelling the input-
quantization error (measured absmax/scale ~8.7e-3 vs 1.5e-2 plain).

The 24-row bottom remainders of 4 consecutive planes merge into one
block-diagonal [120, 96] matmul set.  Weights stay dw-stationary across
the 4 tiles of a plane to amortize the (non-FWL) DoubleRow LDWEIGHTS.

DMA: inputs batched per-plane on the SP-HWDGE ring (stripes across all
16 SDMA engines); outputs stored as fp16 via SWDGE (gpsimd) only --
the ACT-HWDGE ring only stripes across 2 engines and was the previous
bottleneck.  Bias is added during PSUM->SBUF eviction (ScalarE/VectorE
alternating); host upcasts the fp16 output to fp32.
"""
import numpy as np
import ml_dtypes

import concourse.bass as bass
import concourse.tile as tile
from concourse import bacc, mybir
from concourse.bass_utils import run_bass_kernel_spmd

N_CORES = 8
H = W = 512
HP = 518          # 3 + 512 + 3 zero-padded rows
WPB = 528         # padded row width in fp8 bytes (16-aligned; cols 3..514 live)
PS = HP * WPB     # plane stride in elements (fp8 == bytes)
MT = 122          # full-tile output rows (64+64 partition pairs = 128 in-rows)
MB = 24           # bottom remainder rows per plane
GROUP = 4         # planes per merged bottom tile
F8 = ml_dtypes.float8_e4m3fn
DR = mybir.MatmulPerfMode.DoubleRow


def _build_weights(K8):
    """K8 (7,7) f32 (fp8-representable values) -> DoubleRow band packs.

    Full tile [128, 7*256]: for dw, pack at cols [256dw, 256dw+256):
      lhsT[p, i*128+m] = K8[(2(p%64)+i) - m, dw]  (0 <= diff < 7, m < 122)
    partitions 0..63 apply to X8 row pairs, 64..127 to E8 (same band).
    Bottom tile [120, 7*256]: block-diagonal over GROUP planes, 15+15
    partition pairs per plane (30 in-rows), m in [24g, 24g+24).
    """
    p = np.arange(64)[:, None, None]
    i = np.arange(2)[None, :, None]
    m = np.arange(MT)[None, None, :]
    d = 2 * p + i - m
    ok = (d >= 0) & (d < 7)
    wf = np.zeros((128, 7, 2, 128), np.float32)
    for dw in range(7):
        blk = np.zeros((64, 2, MT), np.float32)
        blk[ok] = K8[d[ok], dw]
        wf[0:64, dw, :, :MT] = blk
        wf[64:128, dw, :, :MT] = blk
    pp = np.arange(15)[:, None, None]
    j = np.arange(MB)[None, None, :]
    db = 2 * pp + i - j
    okb = (db >= 0) & (db < 7)
    wb = np.zeros((120, 7, 2, 128), np.float32)
    for dw in range(7):
        blk = np.zeros((15, 2, MB), np.float32)
        blk[okb] = K8[db[okb], dw]
        for g in range(GROUP):
            wb[15 * g:15 * g + 15, dw, :, MB * g:MB * g + MB] = blk
            wb[60 + 15 * g:60 + 15 * g + 15, dw, :, MB * g:MB * g + MB] = blk
    return (wf.reshape(128, 7 * 256).astype(F8),
            wb.reshape(120, 7 * 256).astype(F8))


_NC_CACHE = {}


def _get_module(n_planes: int):
    if n_planes in _NC_CACHE:
        return _NC_CACHE[n_planes]
    assert n_planes % GROUP == 0
    nc = bacc.Bacc("TRN2", target_bir_lowering=False, debug=False,
                   num_devices=N_CORES)
    x8 = nc.dram_tensor("x8", [n_planes, HP, WPB], mybir.dt.float8e4,
                        kind="ExternalInput")
    e8 = nc.dram_tensor("e8", [n_planes, HP, WPB], mybir.dt.float8e4,
                        kind="ExternalInput")
    wf = nc.dram_tensor("wf", [128, 7 * 256], mybir.dt.float8e4,
                        kind="ExternalInput")
    wb = nc.dram_tensor("wb", [120, 7 * 256], mybir.dt.float8e4,
                        kind="ExternalInput")
    bv = nc.dram_tensor("bv", [128, 1], mybir.dt.float32,
                        kind="ExternalInput")
    out = nc.dram_tensor("out", [n_planes, H, W], mybir.dt.float16,
                         kind="ExternalOutput")

    with tile.TileContext(nc) as tc:
        with (
            tc.tile_pool(name="wp", bufs=1) as wpool,
            tc.tile_pool(name="xt", bufs=8) as xpool,
            tc.tile_pool(name="gt", bufs=3) as gpool,
            tc.tile_pool(name="ps", bufs=8, space="PSUM") as pspool,
            tc.tile_pool(name="ob", bufs=10) as obpool,
            tc.tile_pool(name="og", bufs=3) as ogpool,
        ):
            wft = wpool.tile([128, 7 * 256], mybir.dt.float8e4)
            nc.sync.dma_start(wft[:], wf.ap())
            wbt = wpool.tile([120, 7 * 256], mybir.dt.float8e4)
            nc.sync.dma_start(wbt[:], wb.ap())
            btile = wpool.tile([128, 1], mybir.dt.float32)
            nc.sync.dma_start(btile[:], bv.ap())

            wfv = wft[:, :].rearrange("p (dw i m) -> p dw i m", dw=7, i=2)
            wbv = wbt[:, :].rearrange("p (dw i m) -> p dw i m", dw=7, i=2)

            def evict(idx, dst, src, rows):
                if idx % 2 == 0:
                    nc.scalar.activation(
                        dst, src, mybir.ActivationFunctionType.Identity,
                        bias=btile[:rows, :], scale=1.0)
                else:
                    nc.vector.tensor_scalar_add(dst, src, btile[:rows, :])

            ev = 0
            for g0 in range(0, n_planes, GROUP):
                # bottom rows (padded 488..517) of GROUP planes, X8+E8
                gtile = gpool.tile([120, 2 * WPB], mybir.dt.float8e4)
                for g in range(GROUP):
                    nc.sync.dma_start(
                        gtile[15 * g:15 * g + 15, :],
                        bass.AP(x8, (g0 + g) * PS + 488 * WPB,
                                [[2 * WPB, 15], [1, 2 * WPB]]))
                    nc.sync.dma_start(
                        gtile[60 + 15 * g:60 + 15 * g + 15, :],
                        bass.AP(e8, (g0 + g) * PS + 488 * WPB,
                                [[2 * WPB, 15], [1, 2 * WPB]]))
                gv = gtile[:, :].rearrange("p (i w) -> p i w", i=2)
                for g in range(GROUP):
                    p = g0 + g
                    # 4 full tiles: X8 pairs -> partitions 0..63,
                    # E8 pairs -> 64..127 (one tile = 2 rows x 528 B)
                    xt = xpool.tile([128, 4 * 2 * WPB], mybir.dt.float8e4)
                    nc.sync.dma_start(
                        xt[0:64, :].rearrange("p (t b) -> p t b", t=4),
                        bass.AP(x8, p * PS,
                                [[2 * WPB, 64], [MT * WPB, 4], [1, 2 * WPB]]))
                    nc.sync.dma_start(
                        xt[64:128, :].rearrange("p (t b) -> p t b", t=4),
                        bass.AP(e8, p * PS,
                                [[2 * WPB, 64], [MT * WPB, 4], [1, 2 * WPB]]))
                    xv = xt[:, :].rearrange("p (t i w) -> p t i w", t=4, i=2)

                    pts = [pspool.tile([128, W], mybir.dt.float32,
                                       name="pt")
                           for t in range(4)]
                    last = (g == GROUP - 1)
                    if last:
                        ptb = pspool.tile([128, W], mybir.dt.float32,
                                          name="pt")
                    # dw-stationary weights across the plane's tiles
                    for dw in range(7):
                        for t in range(4):
                            nc.tensor.matmul(
                                pts[t][0:MT, :], wfv[:, dw, :, 0:MT],
                                xv[:, t, :, dw:dw + W],
                                start=(dw == 0), stop=(dw == 6), perf_mode=DR)
                        if last:
                            nc.tensor.matmul(
                                ptb[0:GROUP * MB, :], wbv[:, dw, :, 0:GROUP * MB],
                                gv[:, :, dw:dw + W],
                                start=(dw == 0), stop=(dw == 6), perf_mode=DR)

                    ob = obpool.tile([MT, 4 * W], mybir.dt.float16)
                    for t in range(4):
                        evict(ev, ob[:, t * W:(t + 1) * W], pts[t][0:MT, :], MT)
                        ev += 1
                    nc.gpsimd.dma_start(
                        bass.AP(out, p * H * W,
                                [[W, MT], [MT * W, 4], [1, W]]),
                        ob[:].rearrange("p (t w) -> p t w", t=4))
                    if last:
                        og = ogpool.tile([GROUP * MB, W], mybir.dt.float16)
                        evict(ev, og[:], ptb[0:GROUP * MB, :], GROUP * MB)
                        ev += 1
                        for g2 in range(GROUP):
                            nc.gpsimd.dma_start(
                                bass.AP(out, (g0 + g2) * H * W + 488 * W,
                                        [[W, MB], [1, W]]),
                                og[MB * g2:MB * g2 + MB, :])

    nc.compile()
    _NC_CACHE[n_planes] = nc
    return nc


def _prep_inputs(X, K, b, n_cores=N_CORES):
    Keff = np.asarray(K, np.float32).sum(axis=(0, 1))
    K8 = Keff.astype(F8).astype(np.float32)
    wf_np, wb_np = _build_weights(K8)
    bias = np.float32(np.asarray(b).reshape(-1)[0]) * np.float32(
        np.asarray(K).size)
    bv = np.full((128, 1), bias, np.float32)

    Xr = np.asarray(X, np.float32).reshape(-1, H, W)
    n_total = Xr.shape[0]
    X8 = Xr.astype(F8)
    E8 = (Xr - X8.astype(np.float32)).astype(F8)
    Xp = np.zeros((n_total, HP, WPB), F8)
    Ep = np.zeros((n_total, HP, WPB), F8)
    Xp[:, 3:3 + H, 3:3 + W] = X8
    Ep[:, 3:3 + H, 3:3 + W] = E8
    del X8, E8
    per = n_total // n_cores
    in_maps = [
        {"x8": Xp[i * per:(i + 1) * per], "e8": Ep[i * per:(i + 1) * per],
         "wf": wf_np, "wb": wb_np, "bv": bv}
        for i in range(n_cores)
    ]
    return in_maps, per


def kernel(X, K, b):
    in_maps, per = _prep_inputs(X, K, b)
    nc = _get_module(per)
    res = run_bass_kernel_spmd(nc, in_maps, list(range(N_CORES)))
    out = np.concatenate([np.asarray(res.results[i]["out"])
                          for i in range(N_CORES)], axis=0)
    return out.astype(np.float32).reshape(np.asarray(X).shape)
